# revision 8
# baseline (speedup 1.0000x reference)
"""Causal single-head attention [4, 2048, 1024] on 8 TRN2 NeuronCores.

Sharding: pure SPMD, no collectives. core = 2*b + h  (b = batch, h = query
zigzag half). Each core owns 8 query tiles of 128 rows, zigzag-interleaved so
causal work is balanced: h=0 -> global q128-tiles [0,2,4,6,9,11,13,15],
h=1 -> [1,3,5,7,8,10,12,14] (both sum to 68 causal k-tile visits).

v5: the Q projection is folded into the scores: the host precomputes
M = 64*Wq@Wk^T (one f32 matmul over weights only); on device stage 1
computes B = M8^T x (same shape as the K projection) and stage 2 contracts
B against x_q directly, eliminating the 64 Q-projection matmuls and their
16 PSUM copies. fp8 e4m3 DoubleRow everywhere except k-tiles 0-1 of the
V/context path
(kept bf16: the first query tile's outputs are near-copies of single V rows
and dominate max-relative-error). E's fp8 quantization cancels through the
softmax denominator (summed from the same quantized E).

HW-measured matmul cost = fixed ~90ns + moving (213ns bf16 / 107ns fp8-DR
per 512 cols) + a weight load (~180-310ns) paid only when the stationary
operand CHANGES between consecutive matmuls. So every loop is ordered to
keep the stationary fixed across consecutive instructions, interleaving the
PSUM accumulation groups of the moving blocks instead:
  Q proj:  for (m,i): qb=0,1 share w-chunk      (2 psum groups in flight)
  K proj:  for (m,i): kb=0..3 share w-chunk     (4 groups)
  V proj:  for (k,i): fh=0,1 share x-chunk      (2 groups)
  scores:  for (t,i): J=0,1 share kt-chunk      (2 groups)
  context: lo/hi/sm share the E-chunk (already 3-way)
PSUM->SBUF copies alternate DVE/Activation; the final ctx*1/denom scale runs
on Activation (Copy with per-partition scale) to keep DVE off the critical
path.

Scaling: M8 = fp8(64*Wq@Wk^T) (std ~0.67, e4m3-friendly); B PSUM (std ~21)
copied to fp8 unscaled; logits = psum/(64*32) recovered in the exp
activation with scale 2^-11.
Wv8 = fp8(Wv) unscaled (|Wv|<=1/32 sits in e4m3's 2^-6/subnormal range whose
fixed ~2^-10 step matches the scaled variant's top-binade error).

Causal masking: for context chunk j only k-tiles KAV[j]-2, KAV[j]-1 can
straddle the diagonal; each gets a [128,128] data-driven mask (ones/tri or
tri/zeros by zigzag parity), so one program serves both parities.
"""

import os
import sys

os.environ.setdefault("JAX_PLATFORMS", "axon")
for _p in (
    "/root/.axon_site",
    "/root/.axon_site/_ro/trn_rl_repo",
    "/root/.axon_site/_ro/pypackages",
    "/opt/trn_rl_repo",
):
    if os.path.isdir(_p) and _p not in sys.path:
        sys.path.append(_p)

import ml_dtypes
import numpy as np

import concourse.bass as bass  # noqa: F401  (import keeps bass registered)
import concourse.tile as tile
from concourse import bacc, mybir
from concourse.bass_utils import run_bass_kernel_spmd

bf16 = ml_dtypes.bfloat16
f8 = ml_dtypes.float8_e4m3

B, S, D = 4, 2048, 1024
P = 128
N_CORES = 8
W8SCALE = 32.0
MSCALE = 64.0
EXP_SCALE = 1.0 / (MSCALE * 32.0)   # logits = psum * EXP_SCALE
NBF = 2                        # k128-tiles kept bf16 in the V/context path

GSEL = (
    [0, 2, 4, 6, 9, 11, 13, 15],   # h = 0
    [1, 3, 5, 7, 8, 10, 12, 14],   # h = 1
)
KJ = (8, 16)                   # scores k128-tile count per local q512 block
KAV = [2, 4, 6, 8, 10, 12, 14, 16]  # context k128-tile count per local q128
NPAIR = D // (2 * P)           # 4 contraction pair-tiles over d/f


def _emit(nc, tc, reps=1):
    bt = mybir.dt.bfloat16
    e4 = mybir.dt.float8e4

    xt8_d = nc.dram_tensor("xt8", [NPAIR, P, 2, S], e4, kind="ExternalInput").ap()
    xtq8_d = nc.dram_tensor("xtq8", [NPAIR, P, 2, D], e4, kind="ExternalInput").ap()
    m8_d = nc.dram_tensor("m8", [NPAIR, P, 2, D], e4, kind="ExternalInput").ap()
    wv8_d = nc.dram_tensor("wv8", [NPAIR, P, 2, D], e4, kind="ExternalInput").ap()
    xt_d = nc.dram_tensor("xt", [D, NBF * P], bt, kind="ExternalInput").ap()
    wv_d = nc.dram_tensor("wv", [D, D], bt, kind="ExternalInput").ap()
    mask_d = nc.dram_tensor("masks", [P, 16, P], bt, kind="ExternalInput").ap()
    out_d = nc.dram_tensor("out", [D, D], bt, kind="ExternalOutput").ap()

    for _rep in range(reps):
        _emit_once(nc, tc, xt8_d, xtq8_d, m8_d, wv8_d, xt_d, wv_d,
                   mask_d, out_d)


def _emit_once(nc, tc, xt8_d, xtq8_d, m8_d, wv8_d, xt_d, wv_d,
               mask_d, out_d):
    f32 = mybir.dt.float32
    bt = mybir.dt.bfloat16
    e4 = mybir.dt.float8e4
    ND = D // P                # 8
    DR = mybir.MatmulPerfMode.DoubleRow
    Exp = mybir.ActivationFunctionType.Exp
    Copy = mybir.ActivationFunctionType.Copy

    cp_alt = [0]

    def copy_out(dst, src):
        """alternate PSUM->SBUF copies between DVE and Activation"""
        cp_alt[0] ^= 1
        if cp_alt[0]:
            nc.vector.tensor_copy(dst, src)
        else:
            nc.scalar.activation(dst, src, Copy)

    with (
        tc.tile_pool(name="qp", bufs=NPAIR) as qp,
        tc.tile_pool(name="kp", bufs=NPAIR) as kp,
        tc.tile_pool(name="vbp", bufs=NBF) as vbp,
        tc.tile_pool(name="vpp", bufs=S // (2 * P) - 1) as vpp,
        tc.tile_pool(name="ebp", bufs=5) as ebp,
        tc.tile_pool(name="epp", bufs=11) as epp,
        tc.tile_pool(name="op", bufs=4) as op,
        tc.tile_pool(name="smallp", bufs=2) as smallp,
        tc.tile_pool(name="maskp", bufs=1) as maskp,
    ):
        ones = smallp.tile([P, 1], bt, tag="ones")
        nc.vector.memset(ones[:], 1.0)
        ones8 = smallp.tile([P, 2, 1], e4, tag="ones8")
        nc.vector.memset(ones8[:], 1.0)
        masks = maskp.tile([P, 16, P], bt, tag="masks")

        xq8 = [qp.tile([P, 2, D], e4, tag="xq", name=f"xq{i}") for i in range(NPAIR)]
        b8 = [kp.tile([P, 2, S], e4, tag="b8", name=f"b8{i}") for i in range(NPAIR)]
        vvb = [vbp.tile([P, D], bt, tag="vb", name=f"vb{k}") for k in range(NBF)]
        # pair p holds k-tiles 2p, 2p+1 (p >= 1; tiles 0,1 are the bf16 vvb)
        vvp = [None] + [vpp.tile([P, 2, D], e4, tag="vp", name=f"vp{p}")
                        for p in range(1, S // (2 * P))]

        # ---- projections ----
        with (
            tc.tile_pool(name="wp", bufs=2 * NPAIR + ND) as wp,
            tc.tile_pool(name="xp", bufs=NPAIR + ND) as xp,
            tc.tile_pool(name="pp", bufs=8, space="PSUM") as pp,
        ):
            # DMA issue order matters: the first matmul group needs m8+xt8.
            m8t, xt8t = [], []
            for i in range(NPAIR):
                t = wp.tile([P, 2, D], e4, tag="w", name=f"m8{i}")
                nc.sync.dma_start(t[:], m8_d[i])
                m8t.append(t)
                t2 = xp.tile([P, 2, S], e4, tag="x", name=f"xt8{i}")
                nc.sync.dma_start(t2[:], xt8_d[i])
                xt8t.append(t2)
            wv8t = []
            for i in range(NPAIR):
                t = wp.tile([P, 2, D], e4, tag="w", name=f"wv8{i}")
                nc.sync.dma_start(t[:], wv8_d[i])
                wv8t.append(t)
            xtt, wvt = [], []
            for di in range(ND):
                t = xp.tile([P, NBF * P], bt, tag="x", name=f"xt{di}")
                nc.sync.dma_start(t[:], xt_d[P * di:P * (di + 1), :])
                xtt.append(t)
                t2 = wp.tile([P, D], bt, tag="w", name=f"wv{di}")
                nc.sync.dma_start(t2[:], wv_d[P * di:P * (di + 1), :])
                wvt.append(t2)
            nc.sync.dma_start(masks[:], mask_d[:])

            # xtq8 -> persistent xq8 tiles (stage-2 scores moving operand)
            for i in range(NPAIR):
                nc.sync.dma_start(xq8[i][:], xtq8_d[i])

            # B[g, k] = sum_d M8[d, g] xT[d, k]: per (m, i) the M-chunk stays
            # stationary across kb=0..3 (four interleaved PSUM groups)
            for m in range(ND):
                ps = [pp.tile([P, 512], f32, tag="ps", name="psk") for _ in range(4)]
                for i in range(NPAIR):
                    for kb in range(S // 512):
                        nc.tensor.matmul(
                            ps[kb][:],
                            m8t[i][:, :, P * m:P * (m + 1)],
                            xt8t[i][:, :, 512 * kb:512 * (kb + 1)],
                            start=(i == 0), stop=(i == NPAIR - 1),
                            perf_mode=DR,
                        )
                for kb in range(S // 512):
                    copy_out(b8[m // 2][:, m % 2, 512 * kb:512 * (kb + 1)],
                             ps[kb][:])

            # V[k, f]: k-tiles 0..NBF-1 bf16, rest fp8 DoubleRow; per (k, i/di)
            # the x-chunk stays stationary across fh=0,1
            for k in range(S // P):
                ps = [pp.tile([P, 512], f32, tag="ps", name="psv") for _ in range(2)]
                if k < NBF:
                    for di in range(ND):
                        for fh in range(2):
                            nc.tensor.matmul(
                                ps[fh][:],
                                xtt[di][:, P * k:P * (k + 1)],
                                wvt[di][:, 512 * fh:512 * (fh + 1)],
                                start=(di == 0), stop=(di == ND - 1),
                            )
                    for fh in range(2):
                        copy_out(vvb[k][:, 512 * fh:512 * (fh + 1)], ps[fh][:])
                else:
                    for i in range(NPAIR):
                        for fh in range(2):
                            nc.tensor.matmul(
                                ps[fh][:],
                                xt8t[i][:, :, P * k:P * (k + 1)],
                                wv8t[i][:, :, 512 * fh:512 * (fh + 1)],
                                start=(i == 0), stop=(i == NPAIR - 1),
                                perf_mode=DR,
                            )
                    for fh in range(2):
                        copy_out(vvp[k // 2][:, k % 2, 512 * fh:512 * (fh + 1)],
                                 ps[fh][:])

        # ---- attention ----
        with (
            tc.tile_pool(name="sp", bufs=2, space="PSUM") as sp,
            tc.tile_pool(name="cp", bufs=2, space="PSUM") as cp,
            tc.tile_pool(name="zp", bufs=2, space="PSUM") as zp,
            tc.tile_pool(name="rp", bufs=3) as rp,
        ):
            # scores for BOTH q-blocks in one k-sweep: per (t, i) the B-chunk
            # stays stationary across J (two interleaved PSUM groups)
            ebf = {}   # (J, t) -> bf16 E tile, t < NBF
            epr = {}   # (J, p) -> fp8 E pair tile, p >= 1
            for t in range(KJ[1]):
                Js = [J for J in range(2) if t < KJ[J]]
                ps = {J: sp.tile([P, 512], f32, tag="sc", name="sc") for J in Js}
                for i in range(NPAIR):
                    for J in Js:
                        nc.tensor.matmul(
                            ps[J][:],
                            b8[i][:, :, P * t:P * (t + 1)],
                            xq8[i][:, :, 512 * J:512 * (J + 1)],
                            start=(i == 0), stop=(i == NPAIR - 1),
                            perf_mode=DR,
                        )
                for J in Js:
                    if t < NBF:
                        e = ebp.tile([P, 512], bt, tag="e", name=f"e{J}_{t}")
                        ebf[(J, t)] = e
                        edst = e[:]
                    else:
                        if t % 2 == 0:
                            epr[(J, t // 2)] = epp.tile(
                                [P, 2, 512], e4, tag="e8", name=f"e8_{J}_{t // 2}")
                        edst = epr[(J, t // 2)][:, t % 2, :]
                    nc.scalar.activation(edst, ps[J][:], Exp, scale=EXP_SCALE)
                    for c in range(4):
                        j = 4 * J + c
                        if KAV[j] - 2 <= t <= KAV[j] - 1:
                            mslot = masks[:, 2 * j + (t - (KAV[j] - 2)), :]
                            if t < NBF:
                                dst = ebf[(J, t)][:, P * c:P * (c + 1)]
                            else:
                                dst = epr[(J, t // 2)][:, t % 2, P * c:P * (c + 1)]
                            nc.vector.tensor_mul(dst, dst, mslot)

            for J in range(2):
                for c in range(4):
                    j = 4 * J + c
                    n = KAV[j]
                    ctx = cp.tile([P, D], f32, tag="ctx", name="ctx")
                    sm = zp.tile([P, 1], f32, tag="sm", name="sm")
                    last_pair = n // 2 - 1   # 0 -> no fp8 part
                    for t in range(NBF):
                        lhs = ebf[(J, t)][:, P * c:P * (c + 1)]
                        st = (t == 0)
                        sp_ = (t == NBF - 1) and (last_pair < 1)
                        nc.tensor.matmul(ctx[:, 0:512], lhs, vvb[t][:, 0:512],
                                         start=st, stop=sp_)
                        nc.tensor.matmul(ctx[:, 512:1024], lhs,
                                         vvb[t][:, 512:1024], start=st, stop=sp_)
                        nc.tensor.matmul(sm[:], lhs, ones[:], start=st, stop=sp_)
                    for p in range(1, last_pair + 1):
                        lhs = epr[(J, p)][:, :, P * c:P * (c + 1)]
                        sp_ = (p == last_pair)
                        nc.tensor.matmul(ctx[:, 0:512], lhs, vvp[p][:, :, 0:512],
                                         start=False, stop=sp_, perf_mode=DR)
                        nc.tensor.matmul(ctx[:, 512:1024], lhs,
                                         vvp[p][:, :, 512:1024],
                                         start=False, stop=sp_, perf_mode=DR)
                        nc.tensor.matmul(sm[:], lhs, ones8[:],
                                         start=False, stop=sp_, perf_mode=DR)
                    rc = rp.tile([P, 1], f32, tag="rc", name="rc")
                    nc.vector.reciprocal(rc[:], sm[:])
                    o = op.tile([P, D], bt, tag="o", name="o")
                    nc.scalar.activation(o[:], ctx[:], Copy, scale=rc[:])
                    nc.sync.dma_start(out_d[P * j:P * (j + 1), :], o[:])


_CACHE = {}


def _build(reps=1):
    nc = bacc.Bacc(
        "TRN2", target_bir_lowering=False, debug=False,
        enable_asserts=False, num_devices=N_CORES,
    )
    with tile.TileContext(nc) as tc:
        _emit(nc, tc, reps=reps)
    nc.compile()
    return nc


def _pack_pairs(m: np.ndarray) -> np.ndarray:
    """[1024, N] f32 -> [4, 128, 2, N] fp8, d = 256*i + 128*j + p."""
    n = m.shape[1]
    return np.ascontiguousarray(
        m.reshape(NPAIR, 2, P, n).transpose(0, 2, 1, 3)).astype(f8)


def _masks_for(h: int) -> np.ndarray:
    """[128, 16, 128] bf16; chunk j slots (2j, 2j+1) = masks for k-tiles
    KAV[j]-2 and KAV[j]-1. Scores layout [k=part, q=free]: allow k <= q."""
    tri = (np.arange(P)[:, None] <= np.arange(P)[None, :]).astype(np.float32)
    m = np.zeros((P, 16, P), dtype=np.float32)
    for j in range(8):
        g = GSEL[h][j]
        if g == KAV[j] - 1:
            m[:, 2 * j, :] = 1.0
            m[:, 2 * j + 1, :] = tri
        else:
            assert g == KAV[j] - 2
            m[:, 2 * j, :] = tri
            m[:, 2 * j + 1, :] = 0.0
    return m.astype(bf16)


def build_in_maps(x, W_query, W_key, W_value):
    # stage1/stage2 contract A's axis0 with x and axis1 with x_q, i.e. they
    # apply A^T between x_q and x -- so pack A = (Wq Wk^T)^T = Wk Wq^T.
    m = np.asarray(W_key, np.float32) @ np.asarray(W_query, np.float32).T
    m8 = _pack_pairs(m * MSCALE)
    wv8 = _pack_pairs(np.asarray(W_value, np.float32))
    wv = np.asarray(W_value, np.float32).astype(bf16)
    masks = [_masks_for(0), _masks_for(1)]
    in_maps = []
    for core in range(N_CORES):
        b, h = divmod(core, 2)
        xb = np.asarray(x[b], np.float32)
        xbt = np.ascontiguousarray(xb.T)               # [1024, 2048]
        qrows = np.concatenate([np.arange(P * g, P * (g + 1)) for g in GSEL[h]])
        in_maps.append({
            "xt8": _pack_pairs(xbt),
            "xtq8": _pack_pairs(np.ascontiguousarray(xb[qrows].T)),
            "m8": m8, "wv8": wv8,
            "xt": np.ascontiguousarray(xbt[:, :NBF * P]).astype(bf16),
            "wv": wv,
            "masks": masks[h],
        })
    return in_maps


def assemble_out(results) -> np.ndarray:
    out = np.empty((B, S, D), dtype=np.float32)
    for core in range(N_CORES):
        b, h = divmod(core, 2)
        r = np.asarray(results[core]["out"], dtype=np.float32)
        for j, g in enumerate(GSEL[h]):
            out[b, P * g:P * (g + 1), :] = r[P * j:P * (j + 1), :]
    return out


def kernel(x, W_query, W_key, W_value):
    if "nc" not in _CACHE:
        _CACHE["nc"] = _build()
    nc = _CACHE["nc"]
    in_maps = build_in_maps(x, W_query, W_key, W_value)
    r = run_bass_kernel_spmd(nc, in_maps, core_ids=list(range(N_CORES)))
    return assemble_out(r.results)


if __name__ == "__main__":
    rng = np.random.default_rng(0)
    x = rng.standard_normal((B, S, D), dtype=np.float32)
    bound = 1.0 / np.sqrt(D)
    wq = rng.uniform(-bound, bound, (D, D)).astype(np.float32)
    wk = rng.uniform(-bound, bound, (D, D)).astype(np.float32)
    wv = rng.uniform(-bound, bound, (D, D)).astype(np.float32)
    o = kernel(x, wq, wk, wv)
    print("out", o.shape, o.dtype, float(np.abs(o).max()))


# revision 9
# speedup vs baseline: 1.3740x; 1.3740x over previous
"""Causal single-head attention [4, 2048, 1024] on 8 TRN2 NeuronCores.

Sharding: pure SPMD, no collectives. core = 2*b + h  (b = batch, h = query
zigzag half). Each core owns 8 query tiles of 128 rows, zigzag-interleaved so
causal work is balanced: h=0 -> global q128-tiles [0,2,4,6,9,11,13,15],
h=1 -> [1,3,5,7,8,10,12,14] (both sum to 68 causal k-tile visits).

v4: fp8 e4m3 DoubleRow everywhere except k-tiles 0-1 of the V/context path
(kept bf16: the first query tile's outputs are near-copies of single V rows
and dominate max-relative-error). E's fp8 quantization cancels through the
softmax denominator (summed from the same quantized E).

HW-measured matmul cost = fixed ~90ns + moving (213ns bf16 / 107ns fp8-DR
per 512 cols) + a weight load (~180-310ns) paid only when the stationary
operand CHANGES between consecutive matmuls. So every loop is ordered to
keep the stationary fixed across consecutive instructions, interleaving the
PSUM accumulation groups of the moving blocks instead:
  Q proj:  for (m,i): qb=0,1 share w-chunk      (2 psum groups in flight)
  K proj:  for (m,i): kb=0..3 share w-chunk     (4 groups)
  V proj:  for (k,i): fh=0,1 share x-chunk      (2 groups)
  scores:  for (t,i): J=0,1 share kt-chunk      (2 groups)
  context: lo/hi/sm share the E-chunk (already 3-way)
PSUM->SBUF copies alternate DVE/Activation; the final ctx*1/denom scale runs
on Activation (Copy with per-partition scale) to keep DVE off the critical
path.

Scaling: W_q8 = fp8(Wq*32), W_k8 = fp8(Wk*32); QT/KT PSUM (=32*q) copied to
fp8 unscaled; logits recovered in the exp activation with scale 2^-15.
Wv8 = fp8(Wv) unscaled (|Wv|<=1/32 sits in e4m3's 2^-6/subnormal range whose
fixed ~2^-10 step matches the scaled variant's top-binade error).

Causal masking: for context chunk j only k-tiles KAV[j]-2, KAV[j]-1 can
straddle the diagonal; each gets a [128,128] data-driven mask (ones/tri or
tri/zeros by zigzag parity), so one program serves both parities.
"""

import os
import sys

os.environ.setdefault("JAX_PLATFORMS", "axon")
for _p in (
    "/root/.axon_site",
    "/root/.axon_site/_ro/trn_rl_repo",
    "/root/.axon_site/_ro/pypackages",
    "/opt/trn_rl_repo",
):
    if os.path.isdir(_p) and _p not in sys.path:
        sys.path.append(_p)

import ml_dtypes
import numpy as np

import concourse.bass as bass  # noqa: F401  (import keeps bass registered)
import concourse.tile as tile
from concourse import bacc, mybir
from concourse.bass_utils import run_bass_kernel_spmd

bf16 = ml_dtypes.bfloat16
f8 = ml_dtypes.float8_e4m3

B, S, D = 4, 2048, 1024
P = 128
N_CORES = 8
W8SCALE = 32.0
EXP_SCALE = 1.0 / (W8SCALE * W8SCALE * 32.0)   # logits = psum * EXP_SCALE
NBF = 2                        # k128-tiles kept bf16 in the V/context path

GSEL = (
    [0, 2, 4, 6, 9, 11, 13, 15],   # h = 0
    [1, 3, 5, 7, 8, 10, 12, 14],   # h = 1
)
KJ = (8, 16)                   # scores k128-tile count per local q512 block
KAV = [2, 4, 6, 8, 10, 12, 14, 16]  # context k128-tile count per local q128
NPAIR = D // (2 * P)           # 4 contraction pair-tiles over d/f


def _emit(nc, tc, reps=1):
    bt = mybir.dt.bfloat16
    e4 = mybir.dt.float8e4

    xt8_d = nc.dram_tensor("xt8", [NPAIR, P, 2, S], e4, kind="ExternalInput").ap()
    xtq8_d = nc.dram_tensor("xtq8", [NPAIR, P, 2, D], e4, kind="ExternalInput").ap()
    wq8_d = nc.dram_tensor("wq8", [NPAIR, P, 2, D], e4, kind="ExternalInput").ap()
    wk8_d = nc.dram_tensor("wk8", [NPAIR, P, 2, D], e4, kind="ExternalInput").ap()
    wv8_d = nc.dram_tensor("wv8", [NPAIR, P, 2, D], e4, kind="ExternalInput").ap()
    xt_d = nc.dram_tensor("xt", [D, NBF * P], bt, kind="ExternalInput").ap()
    wv_d = nc.dram_tensor("wv", [D, D], bt, kind="ExternalInput").ap()
    mask_d = nc.dram_tensor("masks", [P, 16, P], bt, kind="ExternalInput").ap()
    out_d = nc.dram_tensor("out", [D, D], bt, kind="ExternalOutput").ap()

    for _rep in range(reps):
        _emit_once(nc, tc, xt8_d, xtq8_d, wq8_d, wk8_d, wv8_d, xt_d, wv_d,
                   mask_d, out_d)


def _emit_once(nc, tc, xt8_d, xtq8_d, wq8_d, wk8_d, wv8_d, xt_d, wv_d,
               mask_d, out_d):
    f32 = mybir.dt.float32
    bt = mybir.dt.bfloat16
    e4 = mybir.dt.float8e4
    ND = D // P                # 8
    DR = mybir.MatmulPerfMode.DoubleRow
    Exp = mybir.ActivationFunctionType.Exp
    Copy = mybir.ActivationFunctionType.Copy

    cp_alt = [0]

    def copy_out(dst, src):
        """alternate PSUM->SBUF copies between DVE and Activation"""
        cp_alt[0] ^= 1
        if cp_alt[0]:
            nc.vector.tensor_copy(dst, src)
        else:
            nc.scalar.activation(dst, src, Copy)

    with (
        tc.tile_pool(name="qp", bufs=NPAIR) as qp,
        tc.tile_pool(name="kp", bufs=NPAIR) as kp,
        tc.tile_pool(name="vbp", bufs=NBF) as vbp,
        tc.tile_pool(name="vpp", bufs=S // (2 * P) - 1) as vpp,
        tc.tile_pool(name="ebp", bufs=5) as ebp,
        tc.tile_pool(name="epp", bufs=11) as epp,
        tc.tile_pool(name="op", bufs=4) as op,
        tc.tile_pool(name="smallp", bufs=2) as smallp,
        tc.tile_pool(name="maskp", bufs=1) as maskp,
    ):
        ones = smallp.tile([P, 1], bt, tag="ones")
        nc.vector.memset(ones[:], 1.0)
        ones8 = smallp.tile([P, 2, 1], e4, tag="ones8")
        nc.vector.memset(ones8[:], 1.0)
        masks = maskp.tile([P, 16, P], bt, tag="masks")

        qt8 = [qp.tile([P, 2, D], e4, tag="qt", name=f"qt{i}") for i in range(NPAIR)]
        kt8 = [kp.tile([P, 2, S], e4, tag="kt", name=f"kt{i}") for i in range(NPAIR)]
        vvb = [vbp.tile([P, D], bt, tag="vb", name=f"vb{k}") for k in range(NBF)]
        # pair p holds k-tiles 2p, 2p+1 (p >= 1; tiles 0,1 are the bf16 vvb)
        vvp = [None] + [vpp.tile([P, 2, D], e4, tag="vp", name=f"vp{p}")
                        for p in range(1, S // (2 * P))]

        # ---- projections ----
        with (
            tc.tile_pool(name="wp", bufs=3 * NPAIR + ND) as wp,
            tc.tile_pool(name="xp", bufs=2 * NPAIR + ND) as xp,
            tc.tile_pool(name="pp", bufs=6, space="PSUM") as pp,
        ):
            # DMA issue order matters: the first matmul group needs wq8+xtq8.
            wq8t, xtq8t = [], []
            for i in range(NPAIR):
                t = wp.tile([P, 2, D], e4, tag="w", name=f"wq8{i}")
                nc.sync.dma_start(t[:], wq8_d[i])
                wq8t.append(t)
                t2 = xp.tile([P, 2, D], e4, tag="x", name=f"xtq8{i}")
                nc.sync.dma_start(t2[:], xtq8_d[i])
                xtq8t.append(t2)
            wk8t, xt8t = [], []
            for i in range(NPAIR):
                t = wp.tile([P, 2, D], e4, tag="w", name=f"wk8{i}")
                nc.sync.dma_start(t[:], wk8_d[i])
                wk8t.append(t)
                t2 = xp.tile([P, 2, S], e4, tag="x", name=f"xt8{i}")
                nc.sync.dma_start(t2[:], xt8_d[i])
                xt8t.append(t2)
            wv8t = []
            for i in range(NPAIR):
                t = wp.tile([P, 2, D], e4, tag="w", name=f"wv8{i}")
                nc.sync.dma_start(t[:], wv8_d[i])
                wv8t.append(t)
            xtt, wvt = [], []
            for di in range(ND):
                t = xp.tile([P, NBF * P], bt, tag="x", name=f"xt{di}")
                nc.sync.dma_start(t[:], xt_d[P * di:P * (di + 1), :])
                xtt.append(t)
                t2 = wp.tile([P, D], bt, tag="w", name=f"wv{di}")
                nc.sync.dma_start(t2[:], wv_d[P * di:P * (di + 1), :])
                wvt.append(t2)
            nc.sync.dma_start(masks[:], mask_d[:])

            # QT[f, q] (x32): per (m, i) the w-chunk stays stationary across
            # qb=0,1 (two interleaved PSUM groups)
            for m in range(ND):
                ps = [pp.tile([P, 512], f32, tag="ps", name="psq") for _ in range(2)]
                for i in range(NPAIR):
                    for qb in range(2):
                        nc.tensor.matmul(
                            ps[qb][:],
                            wq8t[i][:, :, P * m:P * (m + 1)],
                            xtq8t[i][:, :, 512 * qb:512 * (qb + 1)],
                            start=(i == 0), stop=(i == NPAIR - 1),
                            perf_mode=DR,
                        )
                for qb in range(2):
                    copy_out(qt8[m // 2][:, m % 2, 512 * qb:512 * (qb + 1)],
                             ps[qb][:])

            # KT[f, k] (x32): per (m, i) w-chunk stationary across kb=0..3
            for m in range(ND):
                ps = [pp.tile([P, 512], f32, tag="ps", name="psk") for _ in range(4)]
                for i in range(NPAIR):
                    for kb in range(S // 512):
                        nc.tensor.matmul(
                            ps[kb][:],
                            wk8t[i][:, :, P * m:P * (m + 1)],
                            xt8t[i][:, :, 512 * kb:512 * (kb + 1)],
                            start=(i == 0), stop=(i == NPAIR - 1),
                            perf_mode=DR,
                        )
                for kb in range(S // 512):
                    copy_out(kt8[m // 2][:, m % 2, 512 * kb:512 * (kb + 1)],
                             ps[kb][:])

            # V[k, f]: k-tiles 0..NBF-1 bf16, rest fp8 DoubleRow; per (k, i/di)
            # the x-chunk stays stationary across fh=0,1
            for k in range(S // P):
                ps = [pp.tile([P, 512], f32, tag="ps", name="psv") for _ in range(2)]
                if k < NBF:
                    for di in range(ND):
                        for fh in range(2):
                            nc.tensor.matmul(
                                ps[fh][:],
                                xtt[di][:, P * k:P * (k + 1)],
                                wvt[di][:, 512 * fh:512 * (fh + 1)],
                                start=(di == 0), stop=(di == ND - 1),
                            )
                    for fh in range(2):
                        copy_out(vvb[k][:, 512 * fh:512 * (fh + 1)], ps[fh][:])
                else:
                    for i in range(NPAIR):
                        for fh in range(2):
                            nc.tensor.matmul(
                                ps[fh][:],
                                xt8t[i][:, :, P * k:P * (k + 1)],
                                wv8t[i][:, :, 512 * fh:512 * (fh + 1)],
                                start=(i == 0), stop=(i == NPAIR - 1),
                                perf_mode=DR,
                            )
                    for fh in range(2):
                        copy_out(vvp[k // 2][:, k % 2, 512 * fh:512 * (fh + 1)],
                                 ps[fh][:])

        # ---- attention ----
        with (
            tc.tile_pool(name="sp", bufs=2, space="PSUM") as sp,
            tc.tile_pool(name="cp", bufs=2, space="PSUM") as cp,
            tc.tile_pool(name="zp", bufs=2, space="PSUM") as zp,
            tc.tile_pool(name="rp", bufs=3) as rp,
        ):
            # scores for BOTH q-blocks in one k-sweep: per (t, i) the kt-chunk
            # stays stationary across J (two interleaved PSUM groups)
            ebf = {}   # (J, t) -> bf16 E tile, t < NBF
            epr = {}   # (J, p) -> fp8 E pair tile, p >= 1
            for t in range(KJ[1]):
                Js = [J for J in range(2) if t < KJ[J]]
                ps = {J: sp.tile([P, 512], f32, tag="sc", name="sc") for J in Js}
                for i in range(NPAIR):
                    for J in Js:
                        nc.tensor.matmul(
                            ps[J][:],
                            kt8[i][:, :, P * t:P * (t + 1)],
                            qt8[i][:, :, 512 * J:512 * (J + 1)],
                            start=(i == 0), stop=(i == NPAIR - 1),
                            perf_mode=DR,
                        )
                for J in Js:
                    if t < NBF:
                        e = ebp.tile([P, 512], bt, tag="e", name=f"e{J}_{t}")
                        ebf[(J, t)] = e
                        edst = e[:]
                    else:
                        if t % 2 == 0:
                            epr[(J, t // 2)] = epp.tile(
                                [P, 2, 512], e4, tag="e8", name=f"e8_{J}_{t // 2}")
                        edst = epr[(J, t // 2)][:, t % 2, :]
                    nc.scalar.activation(edst, ps[J][:], Exp, scale=EXP_SCALE)
                    for c in range(4):
                        j = 4 * J + c
                        if KAV[j] - 2 <= t <= KAV[j] - 1:
                            mslot = masks[:, 2 * j + (t - (KAV[j] - 2)), :]
                            if t < NBF:
                                dst = ebf[(J, t)][:, P * c:P * (c + 1)]
                            else:
                                dst = epr[(J, t // 2)][:, t % 2, P * c:P * (c + 1)]
                            nc.vector.tensor_mul(dst, dst, mslot)

            for J in range(2):
                for c in range(4):
                    j = 4 * J + c
                    n = KAV[j]
                    ctx = cp.tile([P, D], f32, tag="ctx", name="ctx")
                    sm = zp.tile([P, 1], f32, tag="sm", name="sm")
                    last_pair = n // 2 - 1   # 0 -> no fp8 part
                    for t in range(NBF):
                        lhs = ebf[(J, t)][:, P * c:P * (c + 1)]
                        st = (t == 0)
                        sp_ = (t == NBF - 1) and (last_pair < 1)
                        nc.tensor.matmul(ctx[:, 0:512], lhs, vvb[t][:, 0:512],
                                         start=st, stop=sp_)
                        nc.tensor.matmul(ctx[:, 512:1024], lhs,
                                         vvb[t][:, 512:1024], start=st, stop=sp_)
                        nc.tensor.matmul(sm[:], lhs, ones[:], start=st, stop=sp_)
                    for p in range(1, last_pair + 1):
                        lhs = epr[(J, p)][:, :, P * c:P * (c + 1)]
                        sp_ = (p == last_pair)
                        nc.tensor.matmul(ctx[:, 0:512], lhs, vvp[p][:, :, 0:512],
                                         start=False, stop=sp_, perf_mode=DR)
                        nc.tensor.matmul(ctx[:, 512:1024], lhs,
                                         vvp[p][:, :, 512:1024],
                                         start=False, stop=sp_, perf_mode=DR)
                        nc.tensor.matmul(sm[:], lhs, ones8[:],
                                         start=False, stop=sp_, perf_mode=DR)
                    rc = rp.tile([P, 1], f32, tag="rc", name="rc")
                    nc.vector.reciprocal(rc[:], sm[:])
                    o = op.tile([P, D], bt, tag="o", name="o")
                    nc.scalar.activation(o[:], ctx[:], Copy, scale=rc[:])
                    nc.sync.dma_start(out_d[P * j:P * (j + 1), :], o[:])


_CACHE = {}


def _build(reps=1):
    nc = bacc.Bacc(
        "TRN2", target_bir_lowering=False, debug=False,
        enable_asserts=False, num_devices=N_CORES,
    )
    with tile.TileContext(nc) as tc:
        _emit(nc, tc, reps=reps)
    nc.compile()
    return nc


def _pack_pairs(m: np.ndarray) -> np.ndarray:
    """[1024, N] f32 -> [4, 128, 2, N] fp8, d = 256*i + 128*j + p."""
    n = m.shape[1]
    return np.ascontiguousarray(
        m.reshape(NPAIR, 2, P, n).transpose(0, 2, 1, 3)).astype(f8)


def _masks_for(h: int) -> np.ndarray:
    """[128, 16, 128] bf16; chunk j slots (2j, 2j+1) = masks for k-tiles
    KAV[j]-2 and KAV[j]-1. Scores layout [k=part, q=free]: allow k <= q."""
    tri = (np.arange(P)[:, None] <= np.arange(P)[None, :]).astype(np.float32)
    m = np.zeros((P, 16, P), dtype=np.float32)
    for j in range(8):
        g = GSEL[h][j]
        if g == KAV[j] - 1:
            m[:, 2 * j, :] = 1.0
            m[:, 2 * j + 1, :] = tri
        else:
            assert g == KAV[j] - 2
            m[:, 2 * j, :] = tri
            m[:, 2 * j + 1, :] = 0.0
    return m.astype(bf16)


def build_in_maps(x, W_query, W_key, W_value):
    wq8 = _pack_pairs(np.asarray(W_query, np.float32) * W8SCALE)
    wk8 = _pack_pairs(np.asarray(W_key, np.float32) * W8SCALE)
    wv8 = _pack_pairs(np.asarray(W_value, np.float32))
    wv = np.asarray(W_value, np.float32).astype(bf16)
    masks = [_masks_for(0), _masks_for(1)]
    in_maps = []
    for core in range(N_CORES):
        b, h = divmod(core, 2)
        xb = np.asarray(x[b], np.float32)
        xbt = np.ascontiguousarray(xb.T)               # [1024, 2048]
        qrows = np.concatenate([np.arange(P * g, P * (g + 1)) for g in GSEL[h]])
        in_maps.append({
            "xt8": _pack_pairs(xbt),
            "xtq8": _pack_pairs(np.ascontiguousarray(xb[qrows].T)),
            "wq8": wq8, "wk8": wk8, "wv8": wv8,
            "xt": np.ascontiguousarray(xbt[:, :NBF * P]).astype(bf16),
            "wv": wv,
            "masks": masks[h],
        })
    return in_maps


def assemble_out(results) -> np.ndarray:
    out = np.empty((B, S, D), dtype=np.float32)
    for core in range(N_CORES):
        b, h = divmod(core, 2)
        r = np.asarray(results[core]["out"], dtype=np.float32)
        for j, g in enumerate(GSEL[h]):
            out[b, P * g:P * (g + 1), :] = r[P * j:P * (j + 1), :]
    return out


def kernel(x, W_query, W_key, W_value):
    if "nc" not in _CACHE:
        _CACHE["nc"] = _build()
    nc = _CACHE["nc"]
    in_maps = build_in_maps(x, W_query, W_key, W_value)
    r = run_bass_kernel_spmd(nc, in_maps, core_ids=list(range(N_CORES)))
    return assemble_out(r.results)


if __name__ == "__main__":
    rng = np.random.default_rng(0)
    x = rng.standard_normal((B, S, D), dtype=np.float32)
    bound = 1.0 / np.sqrt(D)
    wq = rng.uniform(-bound, bound, (D, D)).astype(np.float32)
    wk = rng.uniform(-bound, bound, (D, D)).astype(np.float32)
    wv = rng.uniform(-bound, bound, (D, D)).astype(np.float32)
    o = kernel(x, wq, wk, wv)
    print("out", o.shape, o.dtype, float(np.abs(o).max()))
